# revision 17
# baseline (speedup 1.0000x reference)
"""Trainium2 Bass kernel for nn_AspectModel (gnn_message_passing).

Data-parallel over batch B=32 across 8 NeuronCores (4 batch items per core).
Everything is computed on-chip per batch item:

  per head h:  qT_h = (Wq_h/sqrt(DK))^T @ x^T          [96+1, S]   (row 96 = ones)
               kT_h = Wk_h^T @ x^T                      [96+1, S]   (row 96 = (fm-1)*1e9)
               scores = qT_h.T @ kT_h                   [S, S]      (col-mask folded in)
               E = exp(scores)  (no max-sub; |score| ~ 2), rowsum via ACT accum
               adj_acc += E * (1/rowsum)
  adj = adj_acc * (fm*0.125) + (1-fm)/512  (row fix + head mean)
  diag(adj) = 1  (affine_select)
  adjT = adj.T  (PE transposes)
  denom = colsum(adjT) + 1  (ones-vector matmul), rden = 1/denom
  L1: AxT: lhsT=x_nat, rhs=adjT -> [D,S];  t1 = relu((AxT.T@W0T + b0) * rden)
  L2: AxT2 from t1;  t2T = relu(W1T.T@AxT2 + b1)  [E,S]
  pooled = max_s(t2T * (amask01*rden))       (division + aspect mask folded)
  FFN on pooled (batched over the core's 4 items at the end).

Matmul operands are float32r end-to-end (1 cyc/row for N>=256, fp32 PSUM).
A post-pass splits multi-wait sync_info into standalone EVSEM instructions
(this walrus build allows 1 sync wait per TPB instruction).
"""

import sys
import numpy as np

if "/opt/trn_rl_repo" not in sys.path:
    sys.path.insert(0, "/opt/trn_rl_repo")

B, S, D, H, DK, L, C = 32, 512, 768, 8, 96, 2, 3
NCORES = 8
BL = B // NCORES          # 4 batch items per core
ST = S // 128             # 4 s-tiles
DTL = D // 128            # 6 d-tiles

_CACHE = {}


def split_waits(nc, max_waits=1):
    """Split multi-wait sync_info into standalone InstEventSemaphore (1 wait
    each) inserted before the owning instruction."""
    from concourse import mybir
    n_split = 0
    for fn in nc.m.functions:
        for blk in fn.blocks:
            out = []
            for inst in blk.instructions:
                si = inst.sync_info
                if si is not None and si.on_wait and len(si.on_wait) > max_waits:
                    waits = list(si.on_wait)
                    keep = waits[-max_waits:]
                    move = waits[:-max_waits]
                    for w in move:
                        ev = mybir.InstEventSemaphore(
                            name=f"{inst.name}-wsplit{n_split}",
                            ins=[], outs=[],
                            sync_info=mybir.SyncInfo(on_wait=[w], on_update=[]),
                        )
                        ev.engine = inst.engine
                        nc.register_instruction(ev)
                        out.append(ev)
                        n_split += 1
                    si.on_wait = keep
                out.append(inst)
            blk.instructions[:] = out
    return n_split


def build_nc():
    import concourse.bass as bass
    import concourse.tile as tile
    from concourse import mybir
    from concourse.masks import make_identity
    from contextlib import ExitStack

    f32 = mybir.dt.float32
    FR = mybir.dt.float32r
    BF = mybir.dt.bfloat16
    AF = mybir.ActivationFunctionType
    OP = mybir.AluOpType

    nc = bass.Bass()

    xT_d = nc.declare_dram_parameter("xT", [BL, D, S], BF, isOutput=False)
    xn_d = nc.declare_dram_parameter("xn", [BL, S, D], BF, isOutput=False)
    fm_d = nc.declare_dram_parameter("fm", [BL, S], f32, isOutput=False)
    fmc_d = nc.declare_dram_parameter("fmc", [BL, 128, ST], f32, isOutput=False)
    am_d = nc.declare_dram_parameter("am", [BL, S], f32, isOutput=False)
    wq_d = nc.declare_dram_parameter("wq", [D, D], BF, isOutput=False)
    wk_d = nc.declare_dram_parameter("wk", [D, D], BF, isOutput=False)
    gw_d = nc.declare_dram_parameter("gw", [L, D, D], BF, isOutput=False)
    gb0_d = nc.declare_dram_parameter("gb0", [1, D], BF, isOutput=False)
    cp_d = nc.declare_dram_parameter("cp", [128, 13], f32, isOutput=False)
    w1T_d = nc.declare_dram_parameter("w1T", [D, D], BF, isOutput=False)
    w2T_d = nc.declare_dram_parameter("w2T", [D, C], BF, isOutput=False)
    out1_d = nc.declare_dram_parameter("out1", [BL, C], f32, isOutput=True)
    out2_d = nc.declare_dram_parameter("out2", [BL, D], f32, isOutput=True)

    with tile.TileContext(nc) as tc:
        with ExitStack() as ctx:
            ep = ctx.enter_context
            wpool = ep(tc.tile_pool(name="wpool", bufs=1))
            xtp = ep(tc.tile_pool(name="xtp", bufs=12))
            xnp = ep(tc.tile_pool(name="xnp", bufs=8))
            rowp = ep(tc.tile_pool(name="rowp", bufs=2))
            qkp = ep(tc.tile_pool(name="qkp", bufs=8))
            expp = ep(tc.tile_pool(name="expp", bufs=8))
            adjp = ep(tc.tile_pool(name="adjp", bufs=8))
            adjTp = ep(tc.tile_pool(name="adjTp", bufs=8))
            t1p = ep(tc.tile_pool(name="t1p", bufs=8))
            axsp = ep(tc.tile_pool(name="axsp", bufs=12))
            smallp = ep(tc.tile_pool(name="smallp", bufs=16))
            colp = ep(tc.tile_pool(name="colp", bufs=8))
            predp = ep(tc.tile_pool(name="predp", bufs=6))
            htp = ep(tc.tile_pool(name="htp", bufs=6))
            outp = ep(tc.tile_pool(name="outp", bufs=1))
            mmps = ep(tc.tile_pool(name="mmps", bufs=4, space="PSUM"))
            tps = ep(tc.tile_pool(name="tps", bufs=2, space="PSUM"))
            sps = ep(tc.tile_pool(name="sps", bufs=2, space="PSUM"))

            # ---- constants / weights (loaded once) ----
            ident = wpool.tile([128, 128], f32, tag="ident")
            make_identity(nc, ident[:, :])
            onesf_row = wpool.tile([1, S], f32, tag="onesf_row")
            nc.vector.memset(onesf_row[:, :], 1.0)
            onesf_col = wpool.tile([128, 1], f32, tag="onesf_col")
            nc.vector.memset(onesf_col[:, :], 1.0)
            ones_row = wpool.tile([1, 128], FR, tag="ones_row")
            nc.scalar.copy(ones_row[:, :], onesf_row[0:1, 0:128])
            ones_row_bf = wpool.tile([1, 128], BF, tag="ones_row_bf")
            nc.vector.memset(ones_row_bf[:, :], 1.0)
            ones_col_bf = wpool.tile([128, 1], BF, tag="ones_col_bf")
            nc.vector.memset(ones_col_bf[:, :], 1.0)

            wq_sb = wpool.tile([128, DTL, D], BF, tag="wq")
            wk_sb = wpool.tile([128, DTL, D], BF, tag="wk")
            for kt in range(DTL):
                nc.sync.dma_start(out=wq_sb[:, kt, :],
                                  in_=wq_d[128 * kt:128 * (kt + 1), :])
                nc.sync.dma_start(out=wk_sb[:, kt, :],
                                  in_=wk_d[128 * kt:128 * (kt + 1), :])

            # prefetch batch-0 activations before the bulk of the weights so
            # the PE can start within ~5us of kernel start
            pre = {}
            for b0 in range(1):
                xt0 = []
                for kt in range(DTL):
                    t = xtp.tile([128, S], BF, tag="xt", name=f"pxt{kt}")
                    nc.sync.dma_start(out=t[:, :], in_=xT_d[0, 128 * kt:128 * (kt + 1), :])
                    xt0.append(t)
                pre["xt"] = xt0
                xn0 = []
                for st in range(ST):
                    t = xnp.tile([128, D], BF, tag="xn", name=f"pxn{st}")
                    nc.sync.dma_start(out=t[:, :], in_=xn_d[0, 128 * st:128 * (st + 1), :])
                    xn0.append(t)
                pre["xn"] = xn0

            gw0_sb = wpool.tile([128, DTL, D], BF, tag="gw0")
            for kt in range(DTL):
                nc.sync.dma_start(out=gw0_sb[:, kt, :],
                                  in_=gw_d[0, 128 * kt:128 * (kt + 1), :])
            gw1_sb = wpool.tile([128, DTL, D], BF, tag="gw1")
            for kt in range(DTL):
                nc.sync.dma_start(out=gw1_sb[:, kt, :],
                                  in_=gw_d[1, 128 * kt:128 * (kt + 1), :])
            w1T_sb = wpool.tile([128, DTL, D], BF, tag="w1T")
            for kt in range(DTL):
                nc.sync.dma_start(out=w1T_sb[:, kt, :],
                                  in_=w1T_d[128 * kt:128 * (kt + 1), :])
            w2T_sb = wpool.tile([128, DTL, C], BF, tag="w2T")
            nc.sync.dma_start(out=w2T_sb[:, :, :],
                              in_=w2T_d[:, :].rearrange("(kt p) m -> p kt m", p=128))
            gb0_sb = wpool.tile([1, D], BF, tag="gb0")
            nc.sync.dma_start(out=gb0_sb[:, :], in_=gb0_d[:, :])
            cp_sb = wpool.tile([128, 13], f32, tag="cp")
            nc.sync.dma_start(out=cp_sb[:, :], in_=cp_d[:, :])

            # preds^T per e-tile, columns = batch items
            predsT = [predp.tile([128, BL], f32, tag="pd", name=f"predsT{i}")
                      for i in range(DTL)]
            predsT_bf = [predp.tile([128, BL], BF, tag="pdb", name=f"predsTb{i}")
                         for i in range(DTL)]

            def heads_phase(b):
                st = {}
                if b == 0:
                    xt = pre["xt"]
                    xn = pre["xn"]
                else:
                    xt = []
                    for kt in range(DTL):
                        t = xtp.tile([128, S], BF, tag="xt", name=f"xt{kt}_{b}")
                        nc.sync.dma_start(out=t[:, :],
                                          in_=xT_d[b, 128 * kt:128 * (kt + 1), :])
                        xt.append(t)
                    xn = []
                    for stt_ in range(ST):
                        t = xnp.tile([128, D], BF, tag="xn", name=f"xn{stt_}_{b}")
                        nc.sync.dma_start(out=t[:, :],
                                          in_=xn_d[b, 128 * stt_:128 * (stt_ + 1), :])
                        xn.append(t)
                st["xt"], st["xn"] = xt, xn
                fm_row = rowp.tile([1, S], f32, tag="fm_row", name=f"fm_{b}")
                am_row = rowp.tile([1, S], f32, tag="am_row", name=f"am_{b}")
                cbi_row = rowp.tile([1, S], f32, tag="cbi_row", name=f"cbi_{b}")
                fmc = colp.tile([128, ST], f32, tag="fmc", name=f"fmc_{b}")
                nc.sync.dma_start(out=fm_row[0:1, :], in_=fm_d[b:b + 1, :])
                nc.sync.dma_start(out=am_row[0:1, :], in_=am_d[b:b + 1, :])
                nc.sync.dma_start(out=fmc[:, :], in_=fmc_d[b, :, :])
                nc.vector.tensor_scalar(cbi_row[0:1, :], fm_row[0:1, :],
                                        1e9, -1e9, OP.mult, OP.add)
                rsc_cols = colp.tile([128, ST], f32, tag="rsc", name=f"rsc_{b}")
                rbi_cols = colp.tile([128, ST], f32, tag="rbi", name=f"rbi_{b}")
                nc.vector.tensor_scalar_mul(rsc_cols[:, :], fmc[:, :], 0.125)
                nc.vector.tensor_scalar(rbi_cols[:, :], fmc[:, :],
                                        -1.0 / 512.0, 1.0 / 512.0, OP.mult, OP.add)
                st["am_row"], st["rsc"], st["rbi"] = am_row, rsc_cols, rbi_cols

                adj = [adjp.tile([128, S], f32, tag="adj", name=f"adj{i}_{b}")
                       for i in range(ST)]
                st["adj"] = adj
                for h in range(H):
                    pq = mmps.tile([128, S], f32, tag="mm")
                    for kt in range(DTL):
                        nc.tensor.matmul(pq[0:96, :],
                                         wq_sb[:, kt, 96 * h:96 * h + 96],
                                         xt[kt][:, :],
                                         start=(kt == 0), stop=(kt == DTL - 1))
                    qt = qkp.tile([128, S], BF, tag="qk")
                    nc.scalar.copy(qt[0:96, :], pq[0:96, :])
                    nc.scalar.copy(qt[96:97, :], onesf_row[0:1, :])

                    pk = mmps.tile([128, S], f32, tag="mm")
                    for kt in range(DTL):
                        nc.tensor.matmul(pk[0:96, :],
                                         wk_sb[:, kt, 96 * h:96 * h + 96],
                                         xt[kt][:, :],
                                         start=(kt == 0), stop=(kt == DTL - 1))
                    kt_t = qkp.tile([128, S], BF, tag="qk")
                    nc.scalar.copy(kt_t[0:96, :], pk[0:96, :])
                    nc.scalar.copy(kt_t[96:97, :], cbi_row[0:1, :])

                    rs4 = smallp.tile([128, ST], f32, tag="rs")
                    ri4 = smallp.tile([128, ST], f32, tag="ri")
                    exps = []
                    for m in range(ST):
                        psc = mmps.tile([128, S], f32, tag="mm")
                        nc.tensor.matmul(psc[:, :],
                                         qt[0:97, 128 * m:128 * (m + 1)],
                                         kt_t[0:97, :])
                        ex = expp.tile([128, S], BF, tag="expb", name=f"ex{m}_{h}_{b}")
                        nc.scalar.activation(ex[:, :], psc[:, :], AF.Exp,
                                             accum_out=rs4[:, m:m + 1])
                        exps.append(ex)
                    nc.vector.reciprocal(ri4[:, :], rs4[:, :])
                    for m in range(ST):
                        if h == 0:
                            nc.vector.tensor_scalar_mul(adj[m][:, :], exps[m][:, :],
                                                        ri4[:, m:m + 1])
                        else:
                            nc.vector.scalar_tensor_tensor(adj[m][:, :], exps[m][:, :],
                                                           ri4[:, m:m + 1], adj[m][:, :],
                                                           OP.mult, OP.add)
                return st

            def gcn_phase(b, stt):
                xt, xn, adj = stt["xt"], stt["xn"], stt["adj"]
                am_row, rsc_cols, rbi_cols = stt["am_row"], stt["rsc"], stt["rbi"]
                rden_row = rowp.tile([1, S], f32, tag="rden", name=f"rden_{b}")
                g_row = rowp.tile([1, S], FR, tag="g_row", name=f"g_{b}")

                adjT = [adjTp.tile([128, S], BF, tag="adjT", name=f"adjT{i}_{b}")
                        for i in range(ST)]
                for mi in range(ST):
                    eng = nc.vector if mi % 2 == 0 else nc.gpsimd
                    eng.tensor_scalar(adj[mi][:, :], adj[mi][:, :],
                                      rsc_cols[:, mi:mi + 1], rbi_cols[:, mi:mi + 1],
                                      OP.mult, OP.add)
                    nc.gpsimd.affine_select(out=adj[mi][:, :], in_=adj[mi][:, :],
                                            compare_op=OP.not_equal, fill=1.0,
                                            base=128 * mi, channel_multiplier=1,
                                            pattern=[[-1, S]])
                    for mj in range(ST):
                        tp = tps.tile([128, 128], f32, tag="tp")
                        nc.tensor.transpose(tp[:, :],
                                            adj[mi][:, 128 * mj:128 * (mj + 1)],
                                            ident[:, :])
                        nc.vector.tensor_copy(adjT[mj][:, 128 * mi:128 * (mi + 1)],
                                              tp[:, :])

                dps = sps.tile([1, S], f32, tag="sm")
                for ktt in range(ST):
                    nc.tensor.matmul(dps[0:1, :], ones_col_bf[0:128, 0:1],
                                     adjT[ktt][:, :],
                                     start=(ktt == 0), stop=(ktt == ST - 1))
                nc.vector.tensor_scalar_add(rden_row[0:1, :], dps[0:1, :], 1.0)
                nc.vector.reciprocal(rden_row[0:1, :], rden_row[0:1, :])
                nc.vector.tensor_mul(g_row[0:1, :], am_row[0:1, :], rden_row[0:1, :])

                rdc = []
                for st_ in range(ST):
                    ps = sps.tile([128, 1], f32, tag="sm")
                    nc.tensor.transpose(ps[:, :],
                                        rden_row[0:1, 128 * st_:128 * (st_ + 1)],
                                        ident[0:1, 0:1])
                    cc = colp.tile([128, 1], f32, tag="rdc", name=f"rdc{st_}_{b}")
                    nc.vector.tensor_copy(cc[:, :], ps[:, :])
                    rdc.append(cc)

                Gps = sps.tile([128, S], f32, tag="sm")
                nc.tensor.matmul(Gps[:, :], ones_row[0:1, 0:128], g_row[0:1, :])
                Gp = expp.tile([128, S], f32, tag="exp", name=f"G_{b}")
                nc.vector.tensor_copy(Gp[:, :], Gps[:, :])

                axs = []
                for mt in range(DTL):
                    pax = mmps.tile([128, S], f32, tag="mm")
                    for ktt in range(ST):
                        nc.tensor.matmul(pax[:, :],
                                         xn[ktt][:, 128 * mt:128 * (mt + 1)],
                                         adjT[ktt][:, :],
                                         start=(ktt == 0), stop=(ktt == ST - 1))
                    a_sb = axsp.tile([128, S], BF, tag="axs", name=f"axs{mt}_{b}")
                    nc.vector.tensor_copy(a_sb[:, :], pax[:, :])
                    axs.append(a_sb)
                t1 = [t1p.tile([128, D], BF, tag="t1", name=f"t1_{i}_{b}")
                      for i in range(ST)]
                NCH = 384
                for st_ in range(ST):
                    for nch in range(2):
                        pw = mmps.tile([128, S], f32, tag="mm")
                        for ktt in range(DTL):
                            nc.tensor.matmul(pw[:, 0:NCH],
                                             axs[ktt][:, 128 * st_:128 * (st_ + 1)],
                                             gw0_sb[:, ktt, NCH * nch:NCH * (nch + 1)],
                                             start=(ktt == 0), stop=False)
                        nc.tensor.matmul(pw[:, 0:NCH], ones_row_bf[0:1, 0:128],
                                         gb0_sb[0:1, NCH * nch:NCH * (nch + 1)],
                                         start=False, stop=True)
                        nc.scalar.activation(t1[st_][:, NCH * nch:NCH * (nch + 1)],
                                             pw[:, 0:NCH], AF.Relu,
                                             scale=rdc[st_][:, 0:1])

                axs2 = []
                for mt in range(DTL):
                    pax = mmps.tile([128, S], f32, tag="mm")
                    for ktt in range(ST):
                        nc.tensor.matmul(pax[:, :],
                                         t1[ktt][:, 128 * mt:128 * (mt + 1)],
                                         adjT[ktt][:, :],
                                         start=(ktt == 0), stop=(ktt == ST - 1))
                    a_sb = axsp.tile([128, S], BF, tag="axs", name=f"axs2_{mt}_{b}")
                    nc.vector.tensor_copy(a_sb[:, :], pax[:, :])
                    axs2.append(a_sb)
                for et in range(DTL):
                    pt2 = mmps.tile([128, S], f32, tag="mm")
                    for ktt in range(DTL):
                        nc.tensor.matmul(pt2[:, :],
                                         gw1_sb[:, ktt, 128 * et:128 * (et + 1)],
                                         axs2[ktt][:, :],
                                         start=(ktt == 0), stop=(ktt == DTL - 1))
                    t2s = expp.tile([128, S], f32, tag="exp", name=f"t2s{et}_{b}")
                    nc.scalar.activation(t2s[:, :], pt2[:, :], AF.Relu,
                                         bias=cp_sb[:, et:et + 1])
                    nc.vector.tensor_mul(t2s[:, :], t2s[:, :], Gp[:, :])
                    nc.vector.tensor_reduce(predsT[et][:, b:b + 1], t2s[:, :],
                                            axis=mybir.AxisListType.X, op=OP.max)

            # software pipeline: heads(b+1) is emitted before gcn(b) so the PE
            # queue has independent matmul work during b's softmax/fix tail
            states = {0: heads_phase(0)}
            for b in range(BL):
                if b + 1 < BL:
                    states[b + 1] = heads_phase(b + 1)
                gcn_phase(b, states.pop(b))

            # ---- FFN over the core's 4 batch items ----
            for et in range(DTL):
                nc.vector.tensor_copy(predsT_bf[et][:, :], predsT[et][:, :])
            hT = []
            for ft in range(DTL):
                ph = sps.tile([128, BL], f32, tag="sm")
                for ktt in range(DTL):
                    nc.tensor.matmul(ph[:, :],
                                     w1T_sb[:, ktt, 128 * ft:128 * (ft + 1)],
                                     predsT_bf[ktt][:, :],
                                     start=(ktt == 0), stop=(ktt == DTL - 1))
                h_sb = htp.tile([128, BL], BF, tag="ht", name=f"hT{ft}")
                nc.scalar.activation(h_sb[:, :], ph[:, :], AF.Relu,
                                     bias=cp_sb[:, 6 + ft:7 + ft])
                hT.append(h_sb)
            ppo = sps.tile([C, BL], f32, tag="sm")
            for ft in range(DTL):
                nc.tensor.matmul(ppo[:, :], w2T_sb[:, ft, 0:C], hT[ft][:, :],
                                 start=(ft == 0), stop=(ft == DTL - 1))
            po_sb = outp.tile([C, BL], f32, tag="po")
            nc.vector.tensor_scalar_add(po_sb[:, :], ppo[:, :], cp_sb[0:C, 12:13])

            # ---- outputs (transpose to natural layout, DMA out) ----
            o2_sb = outp.tile([BL, D], f32, tag="o2")
            for et in range(DTL):
                ps = sps.tile([BL, 128], f32, tag="sm")
                nc.tensor.transpose(ps[:, :], predsT[et][:, :], ident[:, :])
                nc.vector.tensor_copy(o2_sb[:, 128 * et:128 * (et + 1)], ps[:, :])
            nc.sync.dma_start(out=out2_d[:, :], in_=o2_sb[:, :])

            ps1 = sps.tile([BL, C], f32, tag="sm")
            nc.tensor.transpose(ps1[:, :], po_sb[:, :], ident[0:C, 0:C])
            o1_sb = outp.tile([BL, C], f32, tag="o1")
            nc.vector.tensor_copy(o1_sb[:, :], ps1[:, :])
            nc.sync.dma_start(out=out1_d[:, :], in_=o1_sb[:, :])

    split_waits(nc)
    return nc


def prep_in_maps(x, fmask, aspect_mask, wq, wk, gcn_w, gcn_b,
                 ffn_w1, ffn_b1, ffn_w2, ffn_b2):
    """Shard + host-side layout prep (not part of timed HW execution)."""
    import ml_dtypes
    f32 = np.float32
    bf16 = ml_dtypes.bfloat16
    Wq = (wq.transpose(1, 0, 2).reshape(D, H * DK) / np.sqrt(f32(DK))).astype(bf16)
    Wk = wk.transpose(1, 0, 2).reshape(D, H * DK).astype(bf16)
    gwT = np.ascontiguousarray(gcn_w.transpose(0, 2, 1)).astype(bf16)      # [L, d, e]
    gb0 = gcn_b[0].reshape(1, D).astype(bf16)
    w1T = np.ascontiguousarray(ffn_w1.T).astype(bf16)                       # [e, f]
    w2T = np.ascontiguousarray(ffn_w2.T).astype(bf16)                       # [f, C]
    cp = np.zeros((128, 13), f32)
    cp[:, 0:6] = gcn_b[1].reshape(6, 128).T
    cp[:, 6:12] = ffn_b1.reshape(6, 128).T
    cp[0:C, 12] = ffn_b2
    am01 = (aspect_mask == 1).astype(f32)

    in_maps = []
    for c in range(NCORES):
        sl = slice(c * BL, (c + 1) * BL)
        xs = np.ascontiguousarray(x[sl]).astype(bf16)
        in_maps.append({
            "xT": np.ascontiguousarray(xs.transpose(0, 2, 1)),
            "xn": xs,
            "fm": np.ascontiguousarray(fmask[sl]).astype(f32),
            "fmc": np.ascontiguousarray(
                fmask[sl].reshape(BL, ST, 128).transpose(0, 2, 1)).astype(f32),
            "am": np.ascontiguousarray(am01[sl]),
            "wq": Wq, "wk": Wk, "gw": gwT, "gb0": gb0, "cp": cp,
            "w1T": w1T, "w2T": w2T,
        })
    return in_maps


def _get_nc():
    if "nc" not in _CACHE:
        _CACHE["nc"] = build_nc()
    return _CACHE["nc"]


def run_on_hw(in_maps, trace=False, tmpdir=None):
    from concourse.bass_utils import run_bass_kernel_spmd
    return run_bass_kernel_spmd(_get_nc(), in_maps, core_ids=list(range(NCORES)),
                                trace=trace, tmpdir=tmpdir)


def kernel(**inputs):
    in_maps = prep_in_maps(**inputs)
    res = run_on_hw(in_maps).results
    preds_ = np.concatenate([res[c]["out1"] for c in range(NCORES)], axis=0)
    preds = np.concatenate([res[c]["out2"] for c in range(NCORES)], axis=0)
    return preds_.astype(np.float32), preds.astype(np.float32)


# revision 18
# speedup vs baseline: 1.1542x; 1.1542x over previous
"""Trainium2 Bass kernel for nn_AspectModel (gnn_message_passing).

Data-parallel over batch B=32 across 8 NeuronCores (4 batch items per core).
Everything is computed on-chip per batch item:

  per head h:  qT_h = (Wq_h/sqrt(DK))^T @ x^T          [96+1, S]   (row 96 = ones)
               kT_h = Wk_h^T @ x^T                      [96+1, S]   (row 96 = (fm-1)*1e9)
               scores = qT_h.T @ kT_h                   [S, S]      (col-mask folded in)
               E = exp(scores)  (no max-sub; |score| ~ 2), rowsum via ACT accum
               adj_acc += E * (1/rowsum)
  adj = adj_acc * (fm*0.125) + (1-fm)/512  (row fix + head mean)
  diag(adj) = 1  (affine_select)
  adjT = adj.T  (PE transposes)
  denom = colsum(adjT) + 1  (ones-vector matmul), rden = 1/denom
  L1: AxT: lhsT=x_nat, rhs=adjT -> [D,S];  t1 = relu((AxT.T@W0T + b0) * rden)
  L2: AxT2 from t1;  t2T = relu(W1T.T@AxT2 + b1)  [E,S]
  pooled = max_s(t2T * (amask01*rden))       (division + aspect mask folded)
  FFN on pooled (batched over the core's 4 items at the end).

Matmul operands are float32r end-to-end (1 cyc/row for N>=256, fp32 PSUM).
A post-pass splits multi-wait sync_info into standalone EVSEM instructions
(this walrus build allows 1 sync wait per TPB instruction).
"""

import sys
import numpy as np

if "/opt/trn_rl_repo" not in sys.path:
    sys.path.insert(0, "/opt/trn_rl_repo")

B, S, D, H, DK, L, C = 32, 512, 768, 8, 96, 2, 3
NCORES = 8
BL = B // NCORES          # 4 batch items per core
ST = S // 128             # 4 s-tiles
DTL = D // 128            # 6 d-tiles

_CACHE = {}


def split_waits(nc, max_waits=1):
    """Split multi-wait sync_info into standalone InstEventSemaphore (1 wait
    each) inserted before the owning instruction."""
    from concourse import mybir
    n_split = 0
    for fn in nc.m.functions:
        for blk in fn.blocks:
            out = []
            for inst in blk.instructions:
                si = inst.sync_info
                if si is not None and si.on_wait and len(si.on_wait) > max_waits:
                    waits = list(si.on_wait)
                    keep = waits[-max_waits:]
                    move = waits[:-max_waits]
                    for w in move:
                        ev = mybir.InstEventSemaphore(
                            name=f"{inst.name}-wsplit{n_split}",
                            ins=[], outs=[],
                            sync_info=mybir.SyncInfo(on_wait=[w], on_update=[]),
                        )
                        ev.engine = inst.engine
                        nc.register_instruction(ev)
                        out.append(ev)
                        n_split += 1
                    si.on_wait = keep
                out.append(inst)
            blk.instructions[:] = out
    return n_split


def build_nc():
    import concourse.bass as bass
    import concourse.tile as tile
    from concourse import mybir
    from concourse.masks import make_identity
    from contextlib import ExitStack

    f32 = mybir.dt.float32
    FR = mybir.dt.float32r
    BF = mybir.dt.bfloat16
    AF = mybir.ActivationFunctionType
    OP = mybir.AluOpType

    nc = bass.Bass()

    xT_d = nc.declare_dram_parameter("xT", [BL, D, S], BF, isOutput=False)
    xn_d = nc.declare_dram_parameter("xn", [BL, S, D], BF, isOutput=False)
    fm_d = nc.declare_dram_parameter("fm", [BL, S], f32, isOutput=False)
    fmc_d = nc.declare_dram_parameter("fmc", [BL, 128, ST], f32, isOutput=False)
    am_d = nc.declare_dram_parameter("am", [BL, S], f32, isOutput=False)
    wq_d = nc.declare_dram_parameter("wq", [D, D], BF, isOutput=False)
    wk_d = nc.declare_dram_parameter("wk", [D, D], BF, isOutput=False)
    gw_d = nc.declare_dram_parameter("gw", [L, D, D], BF, isOutput=False)
    gb0_d = nc.declare_dram_parameter("gb0", [1, D], BF, isOutput=False)
    cp_d = nc.declare_dram_parameter("cp", [128, 13], f32, isOutput=False)
    w1T_d = nc.declare_dram_parameter("w1T", [D, D], BF, isOutput=False)
    w2T_d = nc.declare_dram_parameter("w2T", [D, C], BF, isOutput=False)
    out1_d = nc.declare_dram_parameter("out1", [BL, C], f32, isOutput=True)
    out2_d = nc.declare_dram_parameter("out2", [BL, D], f32, isOutput=True)

    with tile.TileContext(nc) as tc:
        with ExitStack() as ctx:
            ep = ctx.enter_context
            wpool = ep(tc.tile_pool(name="wpool", bufs=1))
            xtp = ep(tc.tile_pool(name="xtp", bufs=12))
            xnp = ep(tc.tile_pool(name="xnp", bufs=8))
            rowp = ep(tc.tile_pool(name="rowp", bufs=2))
            qkp = ep(tc.tile_pool(name="qkp", bufs=8))
            expp = ep(tc.tile_pool(name="expp", bufs=8))
            adjp = ep(tc.tile_pool(name="adjp", bufs=8))
            adjTp = ep(tc.tile_pool(name="adjTp", bufs=8))
            t1p = ep(tc.tile_pool(name="t1p", bufs=8))
            axsp = ep(tc.tile_pool(name="axsp", bufs=12))
            smallp = ep(tc.tile_pool(name="smallp", bufs=16))
            colp = ep(tc.tile_pool(name="colp", bufs=8))
            predp = ep(tc.tile_pool(name="predp", bufs=6))
            htp = ep(tc.tile_pool(name="htp", bufs=6))
            outp = ep(tc.tile_pool(name="outp", bufs=1))
            mmps = ep(tc.tile_pool(name="mmps", bufs=4, space="PSUM"))
            tps = ep(tc.tile_pool(name="tps", bufs=2, space="PSUM"))
            sps = ep(tc.tile_pool(name="sps", bufs=2, space="PSUM"))

            # ---- constants / weights (loaded once) ----
            ident = wpool.tile([128, 128], f32, tag="ident")
            make_identity(nc, ident[:, :])
            onesf_row = wpool.tile([1, S], f32, tag="onesf_row")
            nc.vector.memset(onesf_row[:, :], 1.0)
            onesf_col = wpool.tile([128, 1], f32, tag="onesf_col")
            nc.vector.memset(onesf_col[:, :], 1.0)
            ones_row = wpool.tile([1, 128], FR, tag="ones_row")
            nc.scalar.copy(ones_row[:, :], onesf_row[0:1, 0:128])
            ones_row_bf = wpool.tile([1, 128], BF, tag="ones_row_bf")
            nc.vector.memset(ones_row_bf[:, :], 1.0)
            ones_col_bf = wpool.tile([128, 1], BF, tag="ones_col_bf")
            nc.vector.memset(ones_col_bf[:, :], 1.0)

            wq_sb = wpool.tile([128, DTL, D], BF, tag="wq")
            wk_sb = wpool.tile([128, DTL, D], BF, tag="wk")
            for kt in range(DTL):
                nc.sync.dma_start(out=wq_sb[:, kt, :],
                                  in_=wq_d[128 * kt:128 * (kt + 1), :])
                nc.sync.dma_start(out=wk_sb[:, kt, :],
                                  in_=wk_d[128 * kt:128 * (kt + 1), :])

            # prefetch batch-0 activations before the bulk of the weights so
            # the PE can start within ~5us of kernel start
            pre = {}
            for b0 in range(1):
                xt0 = []
                for kt in range(DTL):
                    t = xtp.tile([128, S], BF, tag="xt", name=f"pxt{kt}")
                    nc.sync.dma_start(out=t[:, :], in_=xT_d[0, 128 * kt:128 * (kt + 1), :])
                    xt0.append(t)
                pre["xt"] = xt0
                xn0 = []
                for st in range(ST):
                    t = xnp.tile([128, D], BF, tag="xn", name=f"pxn{st}")
                    nc.sync.dma_start(out=t[:, :], in_=xn_d[0, 128 * st:128 * (st + 1), :])
                    xn0.append(t)
                pre["xn"] = xn0

            gw0_sb = wpool.tile([128, DTL, D], BF, tag="gw0")
            for kt in range(DTL):
                nc.sync.dma_start(out=gw0_sb[:, kt, :],
                                  in_=gw_d[0, 128 * kt:128 * (kt + 1), :])
            gw1_sb = wpool.tile([128, DTL, D], BF, tag="gw1")
            for kt in range(DTL):
                nc.sync.dma_start(out=gw1_sb[:, kt, :],
                                  in_=gw_d[1, 128 * kt:128 * (kt + 1), :])
            w1T_sb = wpool.tile([128, DTL, D], BF, tag="w1T")
            for kt in range(DTL):
                nc.sync.dma_start(out=w1T_sb[:, kt, :],
                                  in_=w1T_d[128 * kt:128 * (kt + 1), :])
            w2T_sb = wpool.tile([128, DTL, C], BF, tag="w2T")
            nc.sync.dma_start(out=w2T_sb[:, :, :],
                              in_=w2T_d[:, :].rearrange("(kt p) m -> p kt m", p=128))
            gb0_sb = wpool.tile([1, D], BF, tag="gb0")
            nc.sync.dma_start(out=gb0_sb[:, :], in_=gb0_d[:, :])
            cp_sb = wpool.tile([128, 13], f32, tag="cp")
            nc.sync.dma_start(out=cp_sb[:, :], in_=cp_d[:, :])

            # preds^T per e-tile, columns = batch items
            predsT = [predp.tile([128, BL], f32, tag="pd", name=f"predsT{i}")
                      for i in range(DTL)]
            predsT_bf = [predp.tile([128, BL], BF, tag="pdb", name=f"predsTb{i}")
                         for i in range(DTL)]

            def heads_phase(b, st):
                if b == 0:
                    xt = pre["xt"]
                    xn = pre["xn"]
                else:
                    xt = []
                    for kt in range(DTL):
                        t = xtp.tile([128, S], BF, tag="xt", name=f"xt{kt}_{b}")
                        nc.sync.dma_start(out=t[:, :],
                                          in_=xT_d[b, 128 * kt:128 * (kt + 1), :])
                        xt.append(t)
                    xn = []
                    for stt_ in range(ST):
                        t = xnp.tile([128, D], BF, tag="xn", name=f"xn{stt_}_{b}")
                        nc.sync.dma_start(out=t[:, :],
                                          in_=xn_d[b, 128 * stt_:128 * (stt_ + 1), :])
                        xn.append(t)
                st["xt"], st["xn"] = xt, xn
                fm_row = rowp.tile([1, S], f32, tag="fm_row", name=f"fm_{b}")
                am_row = rowp.tile([1, S], f32, tag="am_row", name=f"am_{b}")
                cbi_row = rowp.tile([1, S], f32, tag="cbi_row", name=f"cbi_{b}")
                fmc = colp.tile([128, ST], f32, tag="fmc", name=f"fmc_{b}")
                nc.sync.dma_start(out=fm_row[0:1, :], in_=fm_d[b:b + 1, :])
                nc.sync.dma_start(out=am_row[0:1, :], in_=am_d[b:b + 1, :])
                nc.sync.dma_start(out=fmc[:, :], in_=fmc_d[b, :, :])
                nc.vector.tensor_scalar(cbi_row[0:1, :], fm_row[0:1, :],
                                        1e9, -1e9, OP.mult, OP.add)
                rsc_cols = colp.tile([128, ST], f32, tag="rsc", name=f"rsc_{b}")
                rbi_cols = colp.tile([128, ST], f32, tag="rbi", name=f"rbi_{b}")
                nc.vector.tensor_scalar_mul(rsc_cols[:, :], fmc[:, :], 0.125)
                nc.vector.tensor_scalar(rbi_cols[:, :], fmc[:, :],
                                        -1.0 / 512.0, 1.0 / 512.0, OP.mult, OP.add)
                st["am_row"], st["rsc"], st["rbi"] = am_row, rsc_cols, rbi_cols

                adj = [adjp.tile([128, S], f32, tag="adj", name=f"adj{i}_{b}")
                       for i in range(ST)]
                st["adj"] = adj
                yield
                for h in range(H):
                    pq = mmps.tile([128, S], f32, tag="mm")
                    for kt in range(DTL):
                        nc.tensor.matmul(pq[0:96, :],
                                         wq_sb[:, kt, 96 * h:96 * h + 96],
                                         xt[kt][:, :],
                                         start=(kt == 0), stop=(kt == DTL - 1))
                    qt = qkp.tile([128, S], BF, tag="qk")
                    nc.scalar.copy(qt[0:96, :], pq[0:96, :])
                    nc.scalar.copy(qt[96:97, :], onesf_row[0:1, :])

                    pk = mmps.tile([128, S], f32, tag="mm")
                    for kt in range(DTL):
                        nc.tensor.matmul(pk[0:96, :],
                                         wk_sb[:, kt, 96 * h:96 * h + 96],
                                         xt[kt][:, :],
                                         start=(kt == 0), stop=(kt == DTL - 1))
                    kt_t = qkp.tile([128, S], BF, tag="qk")
                    nc.scalar.copy(kt_t[0:96, :], pk[0:96, :])
                    nc.scalar.copy(kt_t[96:97, :], cbi_row[0:1, :])

                    rs4 = smallp.tile([128, ST], f32, tag="rs")
                    ri4 = smallp.tile([128, ST], f32, tag="ri")
                    exps = []
                    for m in range(ST):
                        psc = mmps.tile([128, S], f32, tag="mm")
                        nc.tensor.matmul(psc[:, :],
                                         qt[0:97, 128 * m:128 * (m + 1)],
                                         kt_t[0:97, :])
                        ex = expp.tile([128, S], BF, tag="expb", name=f"ex{m}_{h}_{b}")
                        nc.scalar.activation(ex[:, :], psc[:, :], AF.Exp,
                                             accum_out=rs4[:, m:m + 1])
                        exps.append(ex)
                    nc.vector.reciprocal(ri4[:, :], rs4[:, :])
                    for m in range(ST):
                        if h == 0:
                            nc.vector.tensor_scalar_mul(adj[m][:, :], exps[m][:, :],
                                                        ri4[:, m:m + 1])
                        else:
                            nc.vector.scalar_tensor_tensor(adj[m][:, :], exps[m][:, :],
                                                           ri4[:, m:m + 1], adj[m][:, :],
                                                           OP.mult, OP.add)
                    yield

            def gcn_phase(b, stt):
                xt, xn, adj = stt["xt"], stt["xn"], stt["adj"]  # noqa
                am_row, rsc_cols, rbi_cols = stt["am_row"], stt["rsc"], stt["rbi"]
                rden_row = rowp.tile([1, S], f32, tag="rden", name=f"rden_{b}")
                g_row = rowp.tile([1, S], FR, tag="g_row", name=f"g_{b}")

                adjT = [adjTp.tile([128, S], BF, tag="adjT", name=f"adjT{i}_{b}")
                        for i in range(ST)]
                for mi in range(ST):
                    eng = nc.vector if mi % 2 == 0 else nc.gpsimd
                    eng.tensor_scalar(adj[mi][:, :], adj[mi][:, :],
                                      rsc_cols[:, mi:mi + 1], rbi_cols[:, mi:mi + 1],
                                      OP.mult, OP.add)
                    nc.gpsimd.affine_select(out=adj[mi][:, :], in_=adj[mi][:, :],
                                            compare_op=OP.not_equal, fill=1.0,
                                            base=128 * mi, channel_multiplier=1,
                                            pattern=[[-1, S]])
                    for mj in range(ST):
                        tp = tps.tile([128, 128], f32, tag="tp")
                        nc.tensor.transpose(tp[:, :],
                                            adj[mi][:, 128 * mj:128 * (mj + 1)],
                                            ident[:, :])
                        nc.vector.tensor_copy(adjT[mj][:, 128 * mi:128 * (mi + 1)],
                                              tp[:, :])
                    yield

                dps = sps.tile([1, S], f32, tag="sm")
                for ktt in range(ST):
                    nc.tensor.matmul(dps[0:1, :], ones_col_bf[0:128, 0:1],
                                     adjT[ktt][:, :],
                                     start=(ktt == 0), stop=(ktt == ST - 1))
                nc.vector.tensor_scalar_add(rden_row[0:1, :], dps[0:1, :], 1.0)
                nc.vector.reciprocal(rden_row[0:1, :], rden_row[0:1, :])
                nc.vector.tensor_mul(g_row[0:1, :], am_row[0:1, :], rden_row[0:1, :])

                rdc = []
                for st_ in range(ST):
                    ps = sps.tile([128, 1], f32, tag="sm")
                    nc.tensor.transpose(ps[:, :],
                                        rden_row[0:1, 128 * st_:128 * (st_ + 1)],
                                        ident[0:1, 0:1])
                    cc = colp.tile([128, 1], f32, tag="rdc", name=f"rdc{st_}_{b}")
                    nc.vector.tensor_copy(cc[:, :], ps[:, :])
                    rdc.append(cc)

                Gps = sps.tile([128, S], f32, tag="sm")
                nc.tensor.matmul(Gps[:, :], ones_row[0:1, 0:128], g_row[0:1, :])
                Gp = expp.tile([128, S], f32, tag="exp", name=f"G_{b}")
                nc.vector.tensor_copy(Gp[:, :], Gps[:, :])
                yield

                axs = []
                for mt in range(DTL):
                    pax = mmps.tile([128, S], f32, tag="mm")
                    for ktt in range(ST):
                        nc.tensor.matmul(pax[:, :],
                                         xn[ktt][:, 128 * mt:128 * (mt + 1)],
                                         adjT[ktt][:, :],
                                         start=(ktt == 0), stop=(ktt == ST - 1))
                    a_sb = axsp.tile([128, S], BF, tag="axs", name=f"axs{mt}_{b}")
                    nc.vector.tensor_copy(a_sb[:, :], pax[:, :])
                    axs.append(a_sb)
                    yield
                t1 = [t1p.tile([128, D], BF, tag="t1", name=f"t1_{i}_{b}")
                      for i in range(ST)]
                NCH = 384
                for st_ in range(ST):
                    for nch in range(2):
                        pw = mmps.tile([128, S], f32, tag="mm")
                        for ktt in range(DTL):
                            nc.tensor.matmul(pw[:, 0:NCH],
                                             axs[ktt][:, 128 * st_:128 * (st_ + 1)],
                                             gw0_sb[:, ktt, NCH * nch:NCH * (nch + 1)],
                                             start=(ktt == 0), stop=False)
                        nc.tensor.matmul(pw[:, 0:NCH], ones_row_bf[0:1, 0:128],
                                         gb0_sb[0:1, NCH * nch:NCH * (nch + 1)],
                                         start=False, stop=True)
                        nc.scalar.activation(t1[st_][:, NCH * nch:NCH * (nch + 1)],
                                             pw[:, 0:NCH], AF.Relu,
                                             scale=rdc[st_][:, 0:1])
                        yield

                axs2 = []
                for mt in range(DTL):
                    pax = mmps.tile([128, S], f32, tag="mm")
                    for ktt in range(ST):
                        nc.tensor.matmul(pax[:, :],
                                         t1[ktt][:, 128 * mt:128 * (mt + 1)],
                                         adjT[ktt][:, :],
                                         start=(ktt == 0), stop=(ktt == ST - 1))
                    a_sb = axsp.tile([128, S], BF, tag="axs", name=f"axs2_{mt}_{b}")
                    nc.vector.tensor_copy(a_sb[:, :], pax[:, :])
                    axs2.append(a_sb)
                    yield
                for et in range(DTL):
                    pt2 = mmps.tile([128, S], f32, tag="mm")
                    for ktt in range(DTL):
                        nc.tensor.matmul(pt2[:, :],
                                         gw1_sb[:, ktt, 128 * et:128 * (et + 1)],
                                         axs2[ktt][:, :],
                                         start=(ktt == 0), stop=(ktt == DTL - 1))
                    t2s = expp.tile([128, S], f32, tag="exp", name=f"t2s{et}_{b}")
                    nc.scalar.activation(t2s[:, :], pt2[:, :], AF.Relu,
                                         bias=cp_sb[:, et:et + 1])
                    nc.vector.tensor_mul(t2s[:, :], t2s[:, :], Gp[:, :])
                    nc.vector.tensor_reduce(predsT[et][:, b:b + 1], t2s[:, :],
                                            axis=mybir.AxisListType.X, op=OP.max)
                    yield

            # fine-grained software pipeline: weave heads(b+1) units between
            # gcn(b) units so the in-order PE queue always has matmul work
            def drain(g):
                if g is not None:
                    for _ in g:
                        pass

            def weave(hgen, ggen, k=4):
                while True:
                    if hgen is not None:
                        try:
                            next(hgen)
                        except StopIteration:
                            hgen = None
                    for _ in range(k):
                        if ggen is not None:
                            try:
                                next(ggen)
                            except StopIteration:
                                ggen = None
                    if hgen is None and ggen is None:
                        return

            states = {b: {} for b in range(BL)}
            drain(heads_phase(0, states[0]))
            for b in range(BL):
                hgen = heads_phase(b + 1, states[b + 1]) if b + 1 < BL else None
                weave(hgen, gcn_phase(b, states[b]))

            # ---- FFN over the core's 4 batch items ----
            for et in range(DTL):
                nc.vector.tensor_copy(predsT_bf[et][:, :], predsT[et][:, :])
            hT = []
            for ft in range(DTL):
                ph = sps.tile([128, BL], f32, tag="sm")
                for ktt in range(DTL):
                    nc.tensor.matmul(ph[:, :],
                                     w1T_sb[:, ktt, 128 * ft:128 * (ft + 1)],
                                     predsT_bf[ktt][:, :],
                                     start=(ktt == 0), stop=(ktt == DTL - 1))
                h_sb = htp.tile([128, BL], BF, tag="ht", name=f"hT{ft}")
                nc.scalar.activation(h_sb[:, :], ph[:, :], AF.Relu,
                                     bias=cp_sb[:, 6 + ft:7 + ft])
                hT.append(h_sb)
            ppo = sps.tile([C, BL], f32, tag="sm")
            for ft in range(DTL):
                nc.tensor.matmul(ppo[:, :], w2T_sb[:, ft, 0:C], hT[ft][:, :],
                                 start=(ft == 0), stop=(ft == DTL - 1))
            po_sb = outp.tile([C, BL], f32, tag="po")
            nc.vector.tensor_scalar_add(po_sb[:, :], ppo[:, :], cp_sb[0:C, 12:13])

            # ---- outputs (transpose to natural layout, DMA out) ----
            o2_sb = outp.tile([BL, D], f32, tag="o2")
            for et in range(DTL):
                ps = sps.tile([BL, 128], f32, tag="sm")
                nc.tensor.transpose(ps[:, :], predsT[et][:, :], ident[:, :])
                nc.vector.tensor_copy(o2_sb[:, 128 * et:128 * (et + 1)], ps[:, :])
            nc.sync.dma_start(out=out2_d[:, :], in_=o2_sb[:, :])

            ps1 = sps.tile([BL, C], f32, tag="sm")
            nc.tensor.transpose(ps1[:, :], po_sb[:, :], ident[0:C, 0:C])
            o1_sb = outp.tile([BL, C], f32, tag="o1")
            nc.vector.tensor_copy(o1_sb[:, :], ps1[:, :])
            nc.sync.dma_start(out=out1_d[:, :], in_=o1_sb[:, :])

    split_waits(nc)
    return nc


def prep_in_maps(x, fmask, aspect_mask, wq, wk, gcn_w, gcn_b,
                 ffn_w1, ffn_b1, ffn_w2, ffn_b2):
    """Shard + host-side layout prep (not part of timed HW execution)."""
    import ml_dtypes
    f32 = np.float32
    bf16 = ml_dtypes.bfloat16
    Wq = (wq.transpose(1, 0, 2).reshape(D, H * DK) / np.sqrt(f32(DK))).astype(bf16)
    Wk = wk.transpose(1, 0, 2).reshape(D, H * DK).astype(bf16)
    gwT = np.ascontiguousarray(gcn_w.transpose(0, 2, 1)).astype(bf16)      # [L, d, e]
    gb0 = gcn_b[0].reshape(1, D).astype(bf16)
    w1T = np.ascontiguousarray(ffn_w1.T).astype(bf16)                       # [e, f]
    w2T = np.ascontiguousarray(ffn_w2.T).astype(bf16)                       # [f, C]
    cp = np.zeros((128, 13), f32)
    cp[:, 0:6] = gcn_b[1].reshape(6, 128).T
    cp[:, 6:12] = ffn_b1.reshape(6, 128).T
    cp[0:C, 12] = ffn_b2
    am01 = (aspect_mask == 1).astype(f32)

    in_maps = []
    for c in range(NCORES):
        sl = slice(c * BL, (c + 1) * BL)
        xs = np.ascontiguousarray(x[sl]).astype(bf16)
        in_maps.append({
            "xT": np.ascontiguousarray(xs.transpose(0, 2, 1)),
            "xn": xs,
            "fm": np.ascontiguousarray(fmask[sl]).astype(f32),
            "fmc": np.ascontiguousarray(
                fmask[sl].reshape(BL, ST, 128).transpose(0, 2, 1)).astype(f32),
            "am": np.ascontiguousarray(am01[sl]),
            "wq": Wq, "wk": Wk, "gw": gwT, "gb0": gb0, "cp": cp,
            "w1T": w1T, "w2T": w2T,
        })
    return in_maps


def _get_nc():
    if "nc" not in _CACHE:
        _CACHE["nc"] = build_nc()
    return _CACHE["nc"]


def run_on_hw(in_maps, trace=False, tmpdir=None):
    from concourse.bass_utils import run_bass_kernel_spmd
    return run_bass_kernel_spmd(_get_nc(), in_maps, core_ids=list(range(NCORES)),
                                trace=trace, tmpdir=tmpdir)


def kernel(**inputs):
    in_maps = prep_in_maps(**inputs)
    res = run_on_hw(in_maps).results
    preds_ = np.concatenate([res[c]["out1"] for c in range(NCORES)], axis=0)
    preds = np.concatenate([res[c]["out2"] for c in range(NCORES)], axis=0)
    return preds_.astype(np.float32), preds.astype(np.float32)
